# revision 29
# baseline (speedup 1.0000x reference)
"""Trainium2 Bass kernel for nn_PolarOut (segment_reduce).

Data-parallel over nodes across 8 NeuronCores. Per core:
  - bf16 node features stream feature-major through TensorE matmuls
    (bf16 moving operands run at 1 col/cycle and enable FWL fast
    weight loads for the 128-col stationaries in the tail)
  - pA and [pB|pC] live in separate PSUM pools with per-half sigmoid
    and product ops, so each pool's buffer frees as early as possible
    and the next tile's matmuls never wait on the ACT->DVE chain
  - the tail is software-pipelined: iteration t emits mains(t),
    SE/R6+products(t-1), segment-accumulate(t-2), keeping the PE
    stream dense so the HAM clock gate stays at 8/8
  - per-128-node-chunk segment reduction via one-hot matmul accumulating
    into a persistent PSUM (6, AW) window accumulator
Host side: shard + transpose + pack inputs (bf16), then sum per-core
(G,6) partials and do the tiny (G,6)->(G,3,3) assembly in numpy.

All matmul PSUM outputs start at partition 0 (this walrus build rejects
nonzero dst base partitions); partition-packing is achieved by
zero-extended stationary operands + PSUM accumulate, and by ordering the
l=2 irrep blocks as [v=4 | v=0..3] so the K=32 tail lands at partition 0.
"""
import math
import numpy as np
from contextlib import ExitStack

N_CORES = 8
CH = 128        # nodes per chunk (PE contraction dim)
TILE = 512      # nodes per compute tile (4 chunks)
AW = 512        # accumulator column window (graphs per core, padded)

# knobs (test.py may flip these)
TRACE = False
REPS = 1          # repeat the whole pipeline in-NEFF (timing); host divides
LAST_RESULTS = None

_cache = {}

# wconst column layout (GP/R6 carry the 0.5x tanh-gate folding)
C_T1, C_H0X, C_GP, C_H2A, C_W2B, C_SE, C_R6, WCOLS = \
    0, 64, 192, 272, 352, 416, 428, 434
# blob per-tile column layout
B_XS, B_X0, B_X2A, B_X2B, B_MC = 0, 512, 1024, 1536, 1664
USE_TILEPOS = False  # K=32 row-tiled v4 matmuls crash this walrus build


def _to_bf16(a):
    import ml_dtypes
    return np.ascontiguousarray(a, dtype=np.float32).astype(ml_dtypes.bfloat16)


def _build(NT, C, L, w0, reps=1, bias_sr_nz=False, bias6_nz=False,
           biasAG_nz=False):
    import concourse.tile as tile
    from concourse import bacc, mybir

    bf16 = mybir.dt.bfloat16
    f32 = mybir.dt.float32
    AF = mybir.ActivationFunctionType
    OP = mybir.AluOpType

    nc = bacc.Bacc("TRN2", target_bir_lowering=False, debug=False,
                   num_devices=N_CORES)
    blob = nc.dram_tensor("blob", [128, NT * C], bf16, kind="ExternalInput").ap()
    wconst = nc.dram_tensor("wconst", [128, WCOLS], bf16, kind="ExternalInput").ap()
    bconst = nc.dram_tensor("bconst", [128, 16], f32, kind="ExternalInput").ap()
    out = nc.dram_tensor("out", [6, AW], f32, kind="ExternalOutput").ap()

    with tile.TileContext(nc) as tc, ExitStack() as ctx:
        cpool = ctx.enter_context(tc.tile_pool(name="consts", bufs=1))
        inpool = ctx.enter_context(tc.tile_pool(name="inp", bufs=5))
        mid = ctx.enter_context(tc.tile_pool(name="mid", bufs=2))
        opool = ctx.enter_context(tc.tile_pool(name="outp", bufs=1))
        psA = ctx.enter_context(tc.tile_pool(name="psA", bufs=2, space="PSUM"))
        psB = ctx.enter_context(tc.tile_pool(name="psB", bufs=2, space="PSUM"))
        psC = ctx.enter_context(tc.tile_pool(name="psC", bufs=2, space="PSUM"))
        psN = ctx.enter_context(tc.tile_pool(name="psN", bufs=1, space="PSUM"))
        psACC = ctx.enter_context(tc.tile_pool(name="psACC", bufs=1, space="PSUM"))

        wc = cpool.tile([128, WCOLS], bf16)
        bc = cpool.tile([128, 16], f32)
        if reps != 1:
            nc.sync.dma_start(wc[:], wconst[:])
            nc.sync.dma_start(bc[:], bconst[:])
        acc = psACC.tile([6, AW], f32)
        nc.vector.memset(acc[:], 0.0)

        GRP = 4  # tiles per DMA transfer

        def emit_front(t, bt):
            # pA = [t1;h0] pre-act; pB = gate pre-act; pC = h2lin.
            # Separate pools so each consumer waits only on its producer.
            pA = psA.tile([128, TILE], f32, tag="pA")
            pB = psB.tile([80, TILE], f32, tag="pB")
            pC = psC.tile([80, TILE], f32, tag="pC")
            # interleave the pA accumulate pair with pB so the second pA
            # matmul (T1) does not wait on the first one's PSUM drain
            nc.tensor.matmul(pA[:, :], wc[:, C_H0X:C_H0X + 128],
                             bt[:, B_X0:B_X0 + 512], start=True, stop=True)
            nc.tensor.matmul(pB[0:80, :], wc[:, C_GP:C_GP + 80],
                             bt[:, B_X0:B_X0 + 512], start=True, stop=True)
            nc.tensor.matmul(pA[0:64, :], wc[:, C_T1:C_T1 + 64],
                             bt[:, B_XS:B_XS + 512], start=False, stop=True)
            nc.tensor.matmul(pC[0:80, :], wc[:, C_H2A:C_H2A + 80],
                             bt[:, B_X2A:B_X2A + 512], start=True, stop=True)
            # v4 block: the block-diagonal W2B stationary region serves both
            # forms — row-tiled (concurrent 32x32 sub-arrays) or full-K masked
            for q in range(4):
                if USE_TILEPOS:
                    nc.tensor.matmul(pC[0:16, 128 * q:128 * (q + 1)],
                                     wc[32 * q:32 * (q + 1),
                                        C_W2B + 16 * q:C_W2B + 16 * (q + 1)],
                                     bt[32 * q:32 * (q + 1),
                                        B_X2B:B_X2B + 128],
                                     start=False, stop=True,
                                     tile_position=(32 * q, 0))
                else:
                    nc.tensor.matmul(pC[0:16, 128 * q:128 * (q + 1)],
                                     wc[:, C_W2B + 16 * q:C_W2B + 16 * (q + 1)],
                                     bt[:, B_X2B:B_X2B + 128],
                                     start=False, stop=True)

            # ath = [silu(pA) | pC * sigmoid-gate]; gate sigmoid is computed
            # as 0.5*tanh(0.5*g)+0.5 (tanh shares the ACT table set with
            # silu): GP carries the inner 0.5, R6 the outer 0.5, and the +1
            # folds into the DVE scalar_tensor_tensor.
            ath = mid.tile([128, 2 * TILE], bf16, tag="ath")
            th = mid.tile([80, TILE], bf16, tag="th")
            if not biasAG_nz:
                nc.scalar.activation(ath[:, 0:512], pA[:, :], AF.Silu)
                nc.scalar.activation(th[0:80, :], pB[0:80, :], AF.Tanh)
            else:
                nc.scalar.activation(ath[:, 0:512], pA[:, :], AF.Silu,
                                     bias=bc[:, 0:1])
                nc.scalar.activation(th[0:80, :], pB[0:80, :], AF.Tanh,
                                     bias=bc[0:80, 1:2])
            nc.vector.scalar_tensor_tensor(ath[0:80, 512:1024], th[0:80, :],
                                           1.0, pC[0:80, :],
                                           OP.add, OP.mult)
            return {"bt": bt, "ath": ath}

        def emit_mid(t, st):
            # per chunk: srep/e6 node-major via activation-stationary matmuls
            ath = st["ath"]
            pN = psN.tile([128, 4 * 12], f32, tag="pN")
            for c in range(4):
                o = 12 * c
                nc.tensor.matmul(pN[:, o:o + 12],
                                 ath[:, 128 * c:128 * (c + 1)],
                                 wc[:, C_SE:C_SE + 12], start=True, stop=True)
                nc.tensor.matmul(pN[:, o + 6:o + 12],
                                 ath[0:80, 512 + 128 * c:512 + 128 * (c + 1)],
                                 wc[0:80, C_R6:C_R6 + 6], start=False, stop=True)
            pn3 = pN[:].rearrange("p (c f) -> p c f", c=4)
            if bias6_nz:
                for c in range(4):
                    nc.vector.tensor_add(pn3[:, c, 6:12], pn3[:, c, 6:12],
                                         bc[:, 10:16])
            # srep -> SBUF in ONE strided ACT copy (DVE reads max one PSUM
            # operand), then ONE DVE multiply for all four chunks
            srcS = mid.tile([128, 4 * 6], bf16, tag="srcS")
            ss3 = srcS[:].rearrange("p (c f) -> p c f", c=4)
            nc.scalar.activation(ss3[:, :, :], pn3[:, :, 0:6], AF.Copy)
            if bias_sr_nz:
                for c in range(4):
                    nc.vector.tensor_add(ss3[:, c, :], ss3[:, c, :],
                                         bc[:, 4:10])
            srcN = mid.tile([128, 4 * 6], bf16, tag="srcN")
            sn3 = srcN[:].rearrange("p (c f) -> p c f", c=4)
            nc.vector.tensor_tensor(sn3[:, :, :], pn3[:, :, 6:12],
                                    ss3[:, :, :], OP.mult)
            st["srcN"] = srcN

        def emit_acc(t, st):
            srcN, bt = st["srcN"], st["bt"]
            for c in range(4):
                cg = 4 * t + c
                nc.tensor.matmul(acc[0:6, w0[cg]:w0[cg] + L],
                                 srcN[:, 6 * c:6 * (c + 1)],
                                 bt[:, B_MC + L * c:B_MC + L * (c + 1)],
                                 start=False, stop=True)

        def _tile_loop():
            # 2-deep software pipeline: mains(t) | SE/R6+products(t-1) |
            # segment-acc(t-2).
            state = {}
            for t0 in range(0, NT, GRP):
                g = min(GRP, NT - t0)
                btg = inpool.tile([128, GRP * C], bf16, tag="bt")
                if t0 == 0:
                    # split the first transfer per-tile so front(0) only
                    # waits on one tile of data (faster pipeline start); the
                    # small wconst/bconst transfers queue behind tile 0
                    for j in range(g):
                        nc.sync.dma_start(btg[:, j * C:(j + 1) * C],
                                          blob[:, j * C:(j + 1) * C])
                    if reps == 1:
                        nc.sync.dma_start(wc[:], wconst[:])
                        nc.sync.dma_start(bc[:], bconst[:])
                else:
                    nc.sync.dma_start(btg[:, :g * C],
                                      blob[:, t0 * C:(t0 + g) * C])
                for j in range(g):
                    t = t0 + j
                    state[t] = emit_front(t, btg[:, j * C:(j + 1) * C])
                    if t - 1 in state:
                        emit_mid(t - 1, state[t - 1])
                    if t - 2 in state:
                        emit_acc(t - 2, state.pop(t - 2))
            if NT - 1 in state:
                emit_mid(NT - 1, state[NT - 1])
            for t in (NT - 2, NT - 1):
                if t >= 0 and t in state:
                    emit_acc(t, state.pop(t))
        if reps == 1:
            _tile_loop()
        else:
            with tc.For_i(0, reps, 1):
                _tile_loop()

        accS = opool.tile([6, AW], f32)
        nc.vector.tensor_copy(accS[:], acc[:])
        nc.sync.dma_start(out[:], accS[:])

    nc.compile()
    return nc


def kernel(node_scalar, node_equi, batch, n_graphs, W_s1, b_s1, W_s2, b_s2,
           W_e0, b_e0, W_e2, W_g, b_g, W_o0, b_o0, W_o2):
    global LAST_RESULTS
    from concourse.bass_utils import run_bass_kernel_spmd

    node_scalar = np.asarray(node_scalar, dtype=np.float32)
    node_equi = np.asarray(node_equi, dtype=np.float32)
    batch = np.asarray(batch).astype(np.int64)
    G = int(n_graphs)
    W_s1 = np.asarray(W_s1, np.float32); b_s1 = np.asarray(b_s1, np.float32)
    W_s2 = np.asarray(W_s2, np.float32); b_s2 = np.asarray(b_s2, np.float32)
    W_e0 = np.asarray(W_e0, np.float32); b_e0 = np.asarray(b_e0, np.float32)
    W_e2 = np.asarray(W_e2, np.float32)
    W_g = np.asarray(W_g, np.float32); b_g = np.asarray(b_g, np.float32)
    W_o0 = np.asarray(W_o0, np.float32); b_o0 = np.asarray(b_o0, np.float32)
    W_o2 = np.asarray(W_o2, np.float32)

    N = node_scalar.shape[0]
    assert N % N_CORES == 0, N
    PER = N // N_CORES
    NCH = (PER + CH - 1) // CH          # chunks per core
    NT = (NCH + 3) // 4                 # tiles per core
    NPAD = NT * TILE                    # padded nodes per core
    L = 16
    C = B_MC + 4 * L

    # --- segment window planning (shared program, per-core data) ---
    g_first = np.array([batch[k * PER] for k in range(N_CORES)], np.int64)
    w0 = np.zeros(NCH, np.int64)
    need = 0
    for cg in range(NCH):
        lo = min(int(batch[k * PER + cg * CH]) - int(g_first[k])
                 for k in range(N_CORES))
        hi = max(int(batch[min(k * PER + (cg + 1) * CH, (k + 1) * PER) - 1])
                 - int(g_first[k]) for k in range(N_CORES))
        lo &= ~1          # keep matmul dst col offsets even
        w0[cg] = lo
        need = max(need, hi - lo + 1)
    assert need <= L, f"window overflow: need {need} > L={L}"
    assert int((w0 + L).max()) <= AW, "acc width overflow"

    # --- weight folding (host) ---
    # gate sigmoid as 0.5*tanh(0.5*x)+0.5: GP carries the 0.5 input scale
    Wg16 = W_e0 @ W_g                                  # (128, 16)
    GPw = 0.5 * np.concatenate([Wg16, np.tile(Wg16, (1, 4))], axis=1)  # (128, 80)
    bg16 = 0.5 * (W_g.T @ b_e0 + b_g)                  # (16,) pre-scaled
    bg80 = np.concatenate([bg16, np.tile(bg16, 4)])    # (80,)
    # h2lin: rows 0:16 = v4 (from K=32 row-tiled matmuls), 16:80 = v<4
    W2A = np.zeros((128, 80), np.float32)
    for v in range(4):
        W2A[v * 32:(v + 1) * 32, 16 + v * 16:16 + (v + 1) * 16] = W_e2
    W2B = np.zeros((128, 64), np.float32)
    for q in range(4):
        W2B[q * 32:(q + 1) * 32, q * 16:(q + 1) * 16] = W_e2
    Q = np.zeros((2, 6), np.float32)
    Q[0, 0] = 1.0
    Q[1, 1:6] = 1.0
    WsQ = W_s2 @ Q                                     # (64, 6)
    bias_sr = Q.T @ b_s2                               # (6,)
    W6A = np.zeros((64, 6), np.float32)
    W6A[:, 0] = W_o0[:, 0]
    SEw = np.zeros((128, 12), np.float32)
    SEw[0:64, 0:6] = WsQ
    SEw[64:128, 6:12] = W6A
    # R6 carries the outer 0.5 of the tanh-sigmoid identity
    R6 = np.zeros((80, 6), np.float32)
    R6[0:16, 5] = 0.5 * W_o2[:, 0]                     # v=4 block
    for v in range(4):
        R6[16 + v * 16:16 + (v + 1) * 16, 1 + v] = 0.5 * W_o2[:, 0]
    bias6 = np.zeros(6, np.float32)
    bias6[0] = b_o0[0]

    wconst = np.zeros((128, WCOLS), np.float32)
    wconst[:, C_T1:C_T1 + 64] = W_s1
    wconst[:, C_H0X + 64:C_H0X + 128] = W_e0           # cols 0:64 stay zero
    wconst[:, C_GP:C_GP + 80] = GPw
    wconst[:, C_H2A:C_H2A + 80] = W2A                  # cols 0:16 zero
    wconst[:, C_W2B:C_W2B + 64] = W2B
    wconst[:, C_SE:C_SE + 12] = SEw
    wconst[0:80, C_R6:C_R6 + 6] = R6
    wconst = _to_bf16(wconst)

    bconst = np.zeros((128, 16), np.float32)
    bconst[0:64, 0] = b_s1
    bconst[64:128, 0] = b_e0
    bconst[0:80, 1] = bg80
    bconst[:, 4:10] = bias_sr[None, :]
    bconst[:, 10:16] = bias6[None, :]

    # --- per-core blobs ---
    in_maps = []
    for k in range(N_CORES):
        s0, s1 = k * PER, (k + 1) * PER
        xs = np.zeros((128, NPAD), np.float32)
        xs[:, :PER] = node_scalar[s0:s1].T
        x0 = np.zeros((128, NPAD), np.float32)
        x0[:, :PER] = node_equi[s0:s1, 0:128].T
        e2 = node_equi[s0:s1, 320:480].reshape(PER, 32, 5).transpose(2, 1, 0)
        x2a = np.zeros((128, NPAD), np.float32)
        x2a[:, :PER] = e2[:4].reshape(128, PER)
        x2b = np.zeros((32, NPAD), np.float32)
        x2b[:, :PER] = e2[4]
        # pack x2b (32, NT*512) -> (128, NT, 128): partition 32q+m holds
        # nodes 512t+128q+j at column j
        x2bp = x2b.reshape(32, NT, 4, 128).transpose(2, 0, 1, 3).reshape(128, NT, 128)
        # one-hot segment maps, relative to per-chunk windows
        rel = np.full(NPAD, -1, np.int64)
        bloc = batch[s0:s1] - g_first[k]
        for cg in range(NCH):
            a, b = cg * CH, min((cg + 1) * CH, PER)
            rel[a:b] = bloc[a:b] - w0[cg]
        mc = (rel[:, None] == np.arange(L)[None, :]).astype(np.float32)
        mcp = mc.reshape(NT, 4, 128, L).transpose(2, 0, 1, 3).reshape(128, NT, 4 * L)

        blob = np.empty((128, NT, C), np.float32)
        blob[:, :, B_XS:B_XS + 512] = xs.reshape(128, NT, 512)
        blob[:, :, B_X0:B_X0 + 512] = x0.reshape(128, NT, 512)
        blob[:, :, B_X2A:B_X2A + 512] = x2a.reshape(128, NT, 512)
        blob[:, :, B_X2B:B_X2B + 128] = x2bp
        blob[:, :, B_MC:B_MC + 4 * L] = mcp
        blob = _to_bf16(blob.reshape(128, NT * C))
        in_maps.append({"blob": blob, "wconst": wconst, "bconst": bconst})

    # --- build (cached) + run ---
    bias_sr_nz = bool(np.any(bias_sr != 0))
    bias6_nz = bool(np.any(bias6 != 0))
    biasAG_nz = bool(np.any(b_s1 != 0) or np.any(b_e0 != 0) or np.any(bg80 != 0))
    key = (N, G, NT, C, L, REPS, bias_sr_nz, bias6_nz, biasAG_nz,
           USE_TILEPOS, tuple(w0.tolist()))
    if key not in _cache:
        _cache[key] = _build(NT, C, L, w0, REPS, bias_sr_nz, bias6_nz,
                             biasAG_nz)
    nc = _cache[key]

    res = run_bass_kernel_spmd(nc, in_maps, list(range(N_CORES)), trace=TRACE)
    LAST_RESULTS = res

    # --- host unshard: sum windowed partials, assemble 3x3 ---
    polar6 = np.zeros((G + AW, 6), np.float64)
    for k in range(N_CORES):
        o = res.results[k]["out"]                      # (6, AW)
        polar6[g_first[k]:g_first[k] + AW] += o.T.astype(np.float64) / REPS
    polar6 = polar6[:G]

    zero = polar6[:, 0]
    d = polar6[:, 1:6]
    d_norm = np.sqrt((d * d).sum(-1))
    dxy, dyz, dz2, dzx, dx2y2 = d[:, 0], d[:, 1], d[:, 2], d[:, 3], d[:, 4]
    cc = 1.0 / math.sqrt(3.0)
    a00 = zero + cc * (d_norm - dz2) + dx2y2
    a11 = zero + cc * (d_norm - dz2) - dx2y2
    a22 = zero + cc * (d_norm + 2.0 * dz2)
    outm = np.empty((G, 3, 3), np.float64)
    outm[:, 0, 0] = a00; outm[:, 0, 1] = dxy; outm[:, 0, 2] = dzx
    outm[:, 1, 0] = dxy; outm[:, 1, 1] = a11; outm[:, 1, 2] = dyz
    outm[:, 2, 0] = dzx; outm[:, 2, 1] = dyz; outm[:, 2, 2] = a22
    return outm.astype(np.float32)

